# revision 43
# baseline (speedup 1.0000x reference)
"""Trainium2 Bass kernel for nn_Classifier_8418135900320 (retrieval_knn).

Reference computes, for S[i,j] = cos(y_i, z_j):
  top1  = mean_i(argmax_j S[i,j] == i)
  top10 = mean_i(i in top-10 indices of row i)

Both reduce to per-row counting: with cnt[i] = #{j : S[i,j] > S[i,i]},
  top1  = mean(cnt == 0),  top10 = mean(cnt <= 9).

Row-scaling by 1/||y_i|| never changes per-row comparisons, so only Z is
normalized (host side: W = Z/||z_j||) and the device ranks rows of
G[i,j] = y_i . w_j.

Sharding: rows of Y (queries) across 8 cores, W replicated.

Precision: inputs are fp8 e4m3 (scaled by SW/SY to dodge the subnormal
range -- a positive per-matrix scale never changes per-row comparisons),
driving the PE at the fp8 DoubleRow rate.  fp8 dot-product noise is ~0.05
while top-10 decision margins on this data are ~0.01, so the device counts
alone cannot decide near-boundary rows; instead any row whose device count
is <= RECHECK_T is re-ranked exactly on the host during the unshard step.
Rows above the threshold are far outside the top-10 (true top-10 rows
measure <= ~10 on device, a 6x empirical margin).

The per-row comparison threshold (the diagonal score d_i = y8_i . w8_i in
device units) is computed on the HOST from the same quantized fp8 operands
and shipped as a [128, 8] fp32 table.  It matches the device's PSUM value
to ~1e-5 relative (fp32 accumulation-order rounding only); a +-1 count
error on the self-match is absorbed by the recheck margin.  This removes
the identity-mask diag extraction, the diag-alignment W rotation, and all
per-row-tile finish chains from the device program.

Per core: 64 score tiles of [128, 1024] PSUM (2 banks each, 4 in flight),
each from 4 DoubleRow matmuls (kp-outer so the stationary y-tile is shared
by the two 512-wide halves).  Each PSUM tile is counted by exactly ONE
engine -- tiles alternate between Vector (strict is_gt + fused accumulate)
and Scalar (Sign with bias=d, scale=-1, + fused accumulate), (rt+ct)%2 --
so per-tile instruction overhead (PSUM access latency + accumulator read)
is paid once per 1024 columns and each engine carries ~43us << PE ~57us.
Per-row-tile partial counts land in [128, 8, 4] accumulator tables that
are reduced ONCE at the end (2 tensor_reduce + 1 fused mult-add; the
constant offset folds into the host unshard), gathered into a [8, 128]
row-major table by four DVE 32x32 block transposes, and written out with
a single 2D DMA.  A 24-matmul dummy warm-up burst over SBUF scratch keeps
the PE busy through the initial DMA window so the HAM clock-gate reaches
2.4 GHz before (or right as) the first real tile's operands land.  No
GpSimd use anywhere; W streams on the Sync HWDGE queue in 1024-col strips
while Y + diag load on the Scalar HWDGE queue (a DMA ring interleaves
packets across its queue window, so the critical first transfers must not
share a ring with the bulk stream).
"""

import numpy as np

B = 8192
D = 512
NCORES = 8
BL = B // NCORES  # 1024 local rows per core
P = 128           # partitions
KC = D // P       # 4 contraction chunks
RT = BL // P      # 8 row tiles
NW = 512          # matmul moving free dim / PSUM bank width (fp32)
TW = 1024         # score tile width (2 PSUM banks)
CTN = B // TW     # 8 col tiles

_compiled = None


def _build_program():
    import concourse.bass as bass
    import concourse.bacc as bacc
    import concourse.tile as tile
    from concourse import mybir

    f32 = mybir.dt.float32
    f8 = mybir.dt.float8e4
    bf16 = mybir.dt.bfloat16
    AL = mybir.AluOpType
    AF = mybir.ActivationFunctionType
    AX = mybir.AxisListType

    nc = bacc.Bacc("TRN2", target_bir_lowering=False, num_devices=NCORES)

    yt = nc.declare_dram_parameter("yt", [D, BL], f8, isOutput=False)
    wt = nc.declare_dram_parameter("wt", [D, B], f8, isOutput=False)
    dg_d = nc.declare_dram_parameter("dg", [P, RT], f32, isOutput=False)
    cnt_d = nc.declare_dram_parameter("cnt", [P, RT], f32, isOutput=True)

    with tile.TileContext(nc) as tc:
        with (
            tc.tile_pool(name="wpool", bufs=1) as wpool,
            tc.tile_pool(name="ypool", bufs=1) as ypool,
            tc.tile_pool(name="psum", bufs=4, space=bass.MemorySpace.PSUM) as pspool,
            tc.tile_pool(name="persist", bufs=1) as persist,
        ):
            w16 = wpool.tile([P, KC, B], f8)
            y16 = ypool.tile([P, KC, BL], f8)
            dgsb = persist.tile([P, RT], f32)
            cd = persist.tile([P, RT, CTN // 2], f32)
            sa = persist.tile([P, RT, CTN // 2], f32)
            warm8 = persist.tile([P, 2, NW // 2], f8)
            # The compare ops' element outputs are junk (only the fused
            # accumulator matters); writing them to one persistent buffer
            # per engine (in-order WAW on the same engine) avoids the
            # pool-rotation semaphores a rotating scratch pool would add
            # to both compare-engine queues.
            scr_d = persist.tile([P, TW], bf16)
            scr_a = persist.tile([P, TW], bf16)
            cdx = persist.tile([P, 1], f32)

            # Loads: Y halves on the Scalar HWDGE ring (nothing queued
            # behind them, so y01 gets that ring's share immediately and
            # the Scalar engine is free for compare work by ~13us); the W
            # strip stream (4.2 MB) on the Sync HWDGE ring in consumption
            # order with the small 128-descriptor diag table slotted after
            # the first-needed strip chunks.  The rings share the 16 DMA
            # engines, so spreading further only starves the critical
            # first transfers.
            nc.scalar.dma_start(y16[:, 0:2, :], yt[0:2 * P, :])
            nc.scalar.dma_start(y16[:, 2:4, :], yt[2 * P:4 * P, :])
            for c in range(CTN):
                c0, c1 = c * TW, (c + 1) * TW
                for k in range(KC):
                    nc.sync.dma_start(
                        w16[:, k, c0:c1], wt[k * P:(k + 1) * P, c0:c1]
                    )
                if c == 1:
                    # diag table (128 x 32B descriptors) after strips 0-1:
                    # needed by the first compare (~13us), and anything
                    # queued ahead of strip 1 delays the PE's second sweep.
                    nc.sync.dma_start(dgsb[:], dg_d[:])

            # HAM warm-up: the PE clock starts throttled at 1.2 GHz and
            # only reaches 2.4 GHz after ~3.4us of sustained activity.  The
            # first real matmul can't start until its operands land
            # (~3-5us of DMA, run-variable); burn that dead time on dummy
            # matmuls over a memset scratch (output never read) so the
            # ramp overlaps the load instead of the first real tiles.
            # N=256 keeps per-dummy cost low (~135ns warm / ~240ns cold),
            # so the burst end self-tracks the clock state: cold bursts
            # trip the un-throttle mid-burst and the remainder drains
            # quickly, while an already-warm PE clears the whole burst
            # before the data lands either way.
            nc.vector.memset(warm8[:], 0.25)
            warmps = pspool.tile([P, TW], f32, tag="pt")
            for _ in range(24):
                nc.tensor.matmul(
                    warmps[:, 0:NW // 2], warm8[:, :, 0:P], warm8[:],
                    start=True, stop=True,
                    perf_mode=mybir.MatmulPerfMode.DoubleRow,
                )

            nslot = {}

            def emit_tile(rt, ct):
                pt = pspool.tile([P, TW], f32, tag="pt")
                # kp outer so consecutive matmuls share the stationary
                # operand (gives the weight path a reuse window).
                for kp in range(KC // 2):
                    for half in range(TW // NW):
                        col0 = ct * TW + half * NW
                        # fp8 DoubleRow: lhsT [K,2,M], rhs [K,2,N] contract
                        # 256 K per pass at 2 MACs/cell/cycle.
                        nc.tensor.matmul(
                            pt[:, half * NW:(half + 1) * NW],
                            y16[:, 2 * kp:2 * kp + 2, rt * P:(rt + 1) * P],
                            w16[:, 2 * kp:2 * kp + 2, col0:col0 + NW],
                            start=(kp == 0),
                            stop=(kp == KC // 2 - 1),
                            perf_mode=mybir.MatmulPerfMode.DoubleRow,
                        )
                # One engine counts the whole tile: strict is_gt on DVE,
                # sign(d - S) on ACT (count_gt = (TW - sum)/2 per tile;
                # exact ties contribute 0.5 -- absorbed by the recheck
                # margin, as is the +-1 self-match from the host diag).
                eng = (rt + ct) % 2
                j = nslot.setdefault((rt, eng), 0)
                nslot[(rt, eng)] = j + 1
                if eng == 0:
                    if rt == RT - 1 and ct == CTN - 1:
                        # The very last tile gates the tail: split its
                        # compare in two 512-wide halves so half 0 runs
                        # while the final matmuls stream and only ~0.9us
                        # (not ~1.4us) remains after the last one.
                        nc.vector.tensor_scalar(
                            scr_d[:, 0:NW], pt[:, 0:NW],
                            dgsb[:, rt:rt + 1], None,
                            op0=AL.is_gt, op1=AL.add,
                            accum_out=cd[:, rt, j:j + 1],
                        )
                        nc.vector.tensor_scalar(
                            scr_d[:, NW:TW], pt[:, NW:TW],
                            dgsb[:, rt:rt + 1], None,
                            op0=AL.is_gt, op1=AL.add,
                            accum_out=cdx[:],
                        )
                        return
                    nc.vector.tensor_scalar(
                        scr_d[:],
                        pt[:],
                        dgsb[:, rt:rt + 1],
                        None,
                        op0=AL.is_gt,
                        op1=AL.add,
                        accum_out=cd[:, rt, j:j + 1],
                    )
                else:
                    nc.scalar.activation(
                        scr_a[:],
                        pt[:],
                        AF.Sign,
                        bias=dgsb[:, rt:rt + 1],
                        scale=-1.0,
                        accum_out=sa[:, rt, j:j + 1],
                    )

            # ct-outer emission matches W strip arrival order; rt rotation
            # inside each sweep is irrelevant to DMA but spreads the two
            # compare engines' slots evenly.
            c1 = persist.tile([P, RT], f32)
            s1 = persist.tile([P, RT], f32)
            for ct in range(CTN):
                for rt in range(RT):
                    emit_tile(rt, ct)
                if ct == CTN - 2:
                    # Slots 0..2 of every row tile are complete once sweep
                    # ct=6 is emitted; pre-reduce them here so only the
                    # last slot remains on the critical tail.
                    nc.vector.tensor_reduce(
                        c1[:], cd[:, :, 0:CTN // 2 - 1], AX.X, AL.add
                    )
                    nc.vector.tensor_reduce(
                        s1[:], sa[:, :, 0:CTN // 2 - 1], AX.X, AL.add
                    )

            # Final: device emits cnt' = sum(cd) - sum(sa)/2 (the +4*TW/2
            # offset is folded in on the host): fold in the last slot of
            # each table, then one fused mult-add, and DMA the [128, 8]
            # table out directly (128 x 32B descriptors spread over 16 DMA
            # engines cost ~0.4us -- cheaper than transposing first).
            c2 = persist.tile([P, RT], f32)
            nc.vector.tensor_add(c2[:], c1[:], cd[:, :, CTN // 2 - 1])
            nc.vector.tensor_add(c2[:, RT - 1:RT], c2[:, RT - 1:RT], cdx[:])
            s2 = persist.tile([P, RT], f32)
            nc.vector.tensor_add(s2[:], s1[:], sa[:, :, CTN // 2 - 1])
            cnt32 = persist.tile([P, RT], f32)
            nc.vector.scalar_tensor_tensor(
                cnt32[:], s2[:], -0.5, c2[:],
                op0=AL.mult, op1=AL.add,
            )
            nc.sync.dma_start(cnt_d[:], cnt32[:])

    nc.compile()
    return nc


SW = 16.0   # scale factors keep fp8 e4m3 inputs out of the subnormal range;
SY = 4.0    # a positive per-matrix scale never changes per-row comparisons.


def _prep_inputs(Z, Y):
    from concourse import mybir
    f8np = mybir.dt.np(mybir.dt.float8e4)
    Z = np.asarray(Z, dtype=np.float32)
    Y = np.asarray(Y, dtype=np.float32)
    zn = np.sqrt((Z.astype(np.float64) ** 2).sum(axis=1))
    W8 = (Z.astype(np.float64) / zn[:, None] * SW).astype(f8np)
    Y8 = (Y.astype(np.float64) * SY).astype(f8np)
    # Host-side diagonal in device units: d_i = y8_i . w8_i over the
    # quantized operands (matches the device PSUM value to fp32
    # accumulation-order rounding, ~1e-5 relative).
    dg = (W8.astype(np.float64) * Y8.astype(np.float64)).sum(axis=1)
    dg = dg.astype(np.float32)
    wt = np.ascontiguousarray(W8.T)
    in_maps = []
    for c in range(NCORES):
        rows = slice(c * BL, (c + 1) * BL)
        in_maps.append({
            "wt": wt,
            "yt": np.ascontiguousarray(Y8[rows].T),
            "dg": np.ascontiguousarray(dg[rows].reshape(RT, P).T),
        })
    return in_maps


def _run(in_maps, trace=False):
    global _compiled
    if _compiled is None:
        _compiled = _build_program()
    from concourse.bass_utils import run_bass_kernel_spmd
    return run_bass_kernel_spmd(_compiled, in_maps, list(range(NCORES)), trace=trace)


RECHECK_T = 64  # device-count threshold below which a row is re-scored


def _gather_counts(res):
    # Device cnt is [P, RT] (partition-major); local row index is rt*P + p.
    cnt = np.concatenate(
        [np.asarray(res.results[c]["cnt"]).T.reshape(-1) for c in range(NCORES)]
    )
    # Device ships cnt' = sum(cd) - sum(sa)/2; each row has 4 Sign tiles
    # of width TW, so the true count is cnt' + 4*TW/2.
    return cnt + (TW / 2.0) * (CTN // 2)


def kernel(Z, Y):
    in_maps = _prep_inputs(Z, Y)
    res = _run(in_maps)
    cnt = _gather_counts(res)
    # fp8 counts carry ~0.05 dot-product noise; any row the device scores as
    # near-boundary (cnt <= RECHECK_T) is re-ranked exactly.  Rows above the
    # threshold are safely outside top-10 (true top-10 rows have fp8 counts
    # far below it -- verified empirically on this data).
    Zf = np.asarray(Z, dtype=np.float64)
    Yf = np.asarray(Y, dtype=np.float64)
    W = Zf / np.sqrt((Zf ** 2).sum(axis=1))[:, None]
    rows = np.nonzero(cnt <= RECHECK_T)[0]
    if rows.size:
        Gr = Yf[rows] @ W.T
        diag = Gr[np.arange(rows.size), rows]
        exact = (Gr > diag[:, None]).sum(axis=1)  # diag never > itself
        cnt = cnt.copy()
        cnt[rows] = exact
    top1 = np.float32((cnt == 0).mean())
    top10 = np.float32((cnt <= 9).mean())
    return (top1, top10)


# revision 44
# speedup vs baseline: 1.0158x; 1.0158x over previous
"""Trainium2 Bass kernel for nn_Classifier_8418135900320 (retrieval_knn).

Reference computes, for S[i,j] = cos(y_i, z_j):
  top1  = mean_i(argmax_j S[i,j] == i)
  top10 = mean_i(i in top-10 indices of row i)

Both reduce to per-row counting: with cnt[i] = #{j : S[i,j] > S[i,i]},
  top1  = mean(cnt == 0),  top10 = mean(cnt <= 9).

Row-scaling by 1/||y_i|| never changes per-row comparisons, so only Z is
normalized (host side: W = Z/||z_j||) and the device ranks rows of
G[i,j] = y_i . w_j.

Sharding: rows of Y (queries) across 8 cores, W replicated.

Precision: inputs are fp8 e4m3 (scaled by SW/SY to dodge the subnormal
range -- a positive per-matrix scale never changes per-row comparisons),
driving the PE at the fp8 DoubleRow rate.  fp8 dot-product noise is ~0.05
while top-10 decision margins on this data are ~0.01, so the device counts
alone cannot decide near-boundary rows; instead any row whose device count
is <= RECHECK_T is re-ranked exactly on the host during the unshard step.
Rows above the threshold are far outside the top-10 (true top-10 rows
measure <= ~10 on device, a 6x empirical margin).

The per-row comparison threshold (the diagonal score d_i = y8_i . w8_i in
device units) is computed on the HOST from the same quantized fp8 operands
and shipped as a [128, 8] fp32 table.  It matches the device's PSUM value
to ~1e-5 relative (fp32 accumulation-order rounding only); a +-1 count
error on the self-match is absorbed by the recheck margin.  This removes
the identity-mask diag extraction, the diag-alignment W rotation, and all
per-row-tile finish chains from the device program.

Per core: 64 score tiles of [128, 1024] PSUM (2 banks each, 4 in flight),
each from 4 DoubleRow matmuls (kp-outer so the stationary y-tile is shared
by the two 512-wide halves).  Each PSUM tile is counted by exactly ONE
engine -- tiles alternate between Vector (strict is_gt + fused accumulate)
and Scalar (Sign with bias=d, scale=-1, + fused accumulate), (rt+ct)%2 --
so per-tile instruction overhead (PSUM access latency + accumulator read)
is paid once per 1024 columns and each engine carries ~43us << PE ~57us.
Per-row-tile partial counts land in [128, 8, 4] accumulator tables that
are reduced ONCE at the end (2 tensor_reduce + 1 fused mult-add; the
constant offset folds into the host unshard), gathered into a [8, 128]
row-major table by four DVE 32x32 block transposes, and written out with
a single 2D DMA.  A 24-matmul dummy warm-up burst over SBUF scratch keeps
the PE busy through the initial DMA window so the HAM clock-gate reaches
2.4 GHz before (or right as) the first real tile's operands land.  No
GpSimd use anywhere; W streams on the Sync HWDGE queue in 1024-col strips
while Y + diag load on the Scalar HWDGE queue (a DMA ring interleaves
packets across its queue window, so the critical first transfers must not
share a ring with the bulk stream).
"""

import numpy as np

B = 8192
D = 512
NCORES = 8
BL = B // NCORES  # 1024 local rows per core
P = 128           # partitions
KC = D // P       # 4 contraction chunks
RT = BL // P      # 8 row tiles
NW = 512          # matmul moving free dim / PSUM bank width (fp32)
TW = 1024         # score tile width (2 PSUM banks)
CTN = B // TW     # 8 col tiles

_compiled = None


def _build_program():
    import concourse.bass as bass
    import concourse.bacc as bacc
    import concourse.tile as tile
    from concourse import mybir

    f32 = mybir.dt.float32
    f8 = mybir.dt.float8e4
    bf16 = mybir.dt.bfloat16
    AL = mybir.AluOpType
    AF = mybir.ActivationFunctionType
    AX = mybir.AxisListType

    nc = bacc.Bacc("TRN2", target_bir_lowering=False, num_devices=NCORES)

    yt = nc.declare_dram_parameter("yt", [D, BL], f8, isOutput=False)
    wt = nc.declare_dram_parameter("wt", [D, B], f8, isOutput=False)
    dg_d = nc.declare_dram_parameter("dg", [P, RT], f32, isOutput=False)
    cnt_d = nc.declare_dram_parameter("cnt", [P, RT], f32, isOutput=True)

    with tile.TileContext(nc) as tc:
        with (
            tc.tile_pool(name="wpool", bufs=1) as wpool,
            tc.tile_pool(name="ypool", bufs=1) as ypool,
            tc.tile_pool(name="psum", bufs=4, space=bass.MemorySpace.PSUM) as pspool,
            tc.tile_pool(name="persist", bufs=1) as persist,
        ):
            w16 = wpool.tile([P, KC, B], f8)
            y16 = ypool.tile([P, KC, BL], f8)
            dgsb = persist.tile([P, RT], f32)
            cd = persist.tile([P, RT, CTN // 2], f32)
            sa = persist.tile([P, RT, CTN // 2], f32)
            warm8 = persist.tile([P, 2, NW // 2], f8)
            # The compare ops' element outputs are junk (only the fused
            # accumulator matters); writing them to one persistent buffer
            # per engine (in-order WAW on the same engine) avoids the
            # pool-rotation semaphores a rotating scratch pool would add
            # to both compare-engine queues.
            scr_d = persist.tile([P, TW], bf16)
            scr_a = persist.tile([P, TW], bf16)
            cdx = persist.tile([P, 1], f32)

            # Loads: Y halves on the Scalar HWDGE ring (nothing queued
            # behind them, so y01 gets that ring's share immediately and
            # the Scalar engine is free for compare work by ~13us); the W
            # strip stream (4.2 MB) on the Sync HWDGE ring in consumption
            # order with the small 128-descriptor diag table slotted after
            # the first-needed strip chunks.  The rings share the 16 DMA
            # engines, so spreading further only starves the critical
            # first transfers.
            nc.scalar.dma_start(y16[:, 0:2, :], yt[0:2 * P, :])
            nc.scalar.dma_start(y16[:, 2:4, :], yt[2 * P:4 * P, :])
            # diag table (128 x 32B descriptors) last on the Scalar ring:
            # lands ~11us -- right before the first compare needs it --
            # without taking any bandwidth from the W strip stream (on the
            # sync ring it delays either strip 1 or the DVE pipeline fill).
            nc.scalar.dma_start(dgsb[:], dg_d[:])
            for c in range(CTN):
                c0, c1 = c * TW, (c + 1) * TW
                for k in range(KC):
                    nc.sync.dma_start(
                        w16[:, k, c0:c1], wt[k * P:(k + 1) * P, c0:c1]
                    )

            # HAM warm-up: the PE clock starts throttled at 1.2 GHz and
            # only reaches 2.4 GHz after ~3.4us of sustained activity.  The
            # first real matmul can't start until its operands land
            # (~3-5us of DMA, run-variable); burn that dead time on dummy
            # matmuls over a memset scratch (output never read) so the
            # ramp overlaps the load instead of the first real tiles.
            # N=256 keeps per-dummy cost low (~135ns warm / ~240ns cold),
            # so the burst end self-tracks the clock state: cold bursts
            # trip the un-throttle mid-burst and the remainder drains
            # quickly, while an already-warm PE clears the whole burst
            # before the data lands either way.
            nc.vector.memset(warm8[:], 0.25)
            warmps = pspool.tile([P, TW], f32, tag="pt")
            for _ in range(24):
                nc.tensor.matmul(
                    warmps[:, 0:NW // 2], warm8[:, :, 0:P], warm8[:],
                    start=True, stop=True,
                    perf_mode=mybir.MatmulPerfMode.DoubleRow,
                )

            nslot = {}

            def emit_tile(rt, ct):
                pt = pspool.tile([P, TW], f32, tag="pt")
                # kp outer so consecutive matmuls share the stationary
                # operand (gives the weight path a reuse window).
                for kp in range(KC // 2):
                    for half in range(TW // NW):
                        col0 = ct * TW + half * NW
                        # fp8 DoubleRow: lhsT [K,2,M], rhs [K,2,N] contract
                        # 256 K per pass at 2 MACs/cell/cycle.
                        nc.tensor.matmul(
                            pt[:, half * NW:(half + 1) * NW],
                            y16[:, 2 * kp:2 * kp + 2, rt * P:(rt + 1) * P],
                            w16[:, 2 * kp:2 * kp + 2, col0:col0 + NW],
                            start=(kp == 0),
                            stop=(kp == KC // 2 - 1),
                            perf_mode=mybir.MatmulPerfMode.DoubleRow,
                        )
                # One engine counts the whole tile: strict is_gt on DVE,
                # sign(d - S) on ACT (count_gt = (TW - sum)/2 per tile;
                # exact ties contribute 0.5 -- absorbed by the recheck
                # margin, as is the +-1 self-match from the host diag).
                eng = (rt + ct) % 2
                j = nslot.setdefault((rt, eng), 0)
                nslot[(rt, eng)] = j + 1
                if eng == 0:
                    if rt == RT - 1 and ct == CTN - 1:
                        # The very last tile gates the tail: split its
                        # compare in two 512-wide halves so half 0 runs
                        # while the final matmuls stream and only ~0.9us
                        # (not ~1.4us) remains after the last one.
                        nc.vector.tensor_scalar(
                            scr_d[:, 0:NW], pt[:, 0:NW],
                            dgsb[:, rt:rt + 1], None,
                            op0=AL.is_gt, op1=AL.add,
                            accum_out=cd[:, rt, j:j + 1],
                        )
                        nc.vector.tensor_scalar(
                            scr_d[:, NW:TW], pt[:, NW:TW],
                            dgsb[:, rt:rt + 1], None,
                            op0=AL.is_gt, op1=AL.add,
                            accum_out=cdx[:],
                        )
                        return
                    nc.vector.tensor_scalar(
                        scr_d[:],
                        pt[:],
                        dgsb[:, rt:rt + 1],
                        None,
                        op0=AL.is_gt,
                        op1=AL.add,
                        accum_out=cd[:, rt, j:j + 1],
                    )
                else:
                    nc.scalar.activation(
                        scr_a[:],
                        pt[:],
                        AF.Sign,
                        bias=dgsb[:, rt:rt + 1],
                        scale=-1.0,
                        accum_out=sa[:, rt, j:j + 1],
                    )

            # ct-outer emission matches W strip arrival order; rt rotation
            # inside each sweep is irrelevant to DMA but spreads the two
            # compare engines' slots evenly.
            c1 = persist.tile([P, RT], f32)
            s1 = persist.tile([P, RT], f32)
            for ct in range(CTN):
                for rt in range(RT):
                    emit_tile(rt, ct)
                if ct == CTN - 2:
                    # Slots 0..2 of every row tile are complete once sweep
                    # ct=6 is emitted; pre-reduce them here so only the
                    # last slot remains on the critical tail.
                    nc.vector.tensor_reduce(
                        c1[:], cd[:, :, 0:CTN // 2 - 1], AX.X, AL.add
                    )
                    nc.vector.tensor_reduce(
                        s1[:], sa[:, :, 0:CTN // 2 - 1], AX.X, AL.add
                    )

            # Final: device emits cnt' = sum(cd) - sum(sa)/2 (the +4*TW/2
            # offset is folded in on the host): fold in the last slot of
            # each table, then one fused mult-add, and DMA the [128, 8]
            # table out directly (128 x 32B descriptors spread over 16 DMA
            # engines cost ~0.4us -- cheaper than transposing first).
            c2 = persist.tile([P, RT], f32)
            nc.vector.tensor_add(c2[:], c1[:], cd[:, :, CTN // 2 - 1])
            nc.vector.tensor_add(c2[:, RT - 1:RT], c2[:, RT - 1:RT], cdx[:])
            s2 = persist.tile([P, RT], f32)
            nc.vector.tensor_add(s2[:], s1[:], sa[:, :, CTN // 2 - 1])
            cnt32 = persist.tile([P, RT], f32)
            nc.vector.scalar_tensor_tensor(
                cnt32[:], s2[:], -0.5, c2[:],
                op0=AL.mult, op1=AL.add,
            )
            nc.sync.dma_start(cnt_d[:], cnt32[:])

    nc.compile()
    return nc


SW = 16.0   # scale factors keep fp8 e4m3 inputs out of the subnormal range;
SY = 4.0    # a positive per-matrix scale never changes per-row comparisons.


def _prep_inputs(Z, Y):
    from concourse import mybir
    f8np = mybir.dt.np(mybir.dt.float8e4)
    Z = np.asarray(Z, dtype=np.float32)
    Y = np.asarray(Y, dtype=np.float32)
    zn = np.sqrt((Z.astype(np.float64) ** 2).sum(axis=1))
    W8 = (Z.astype(np.float64) / zn[:, None] * SW).astype(f8np)
    Y8 = (Y.astype(np.float64) * SY).astype(f8np)
    # Host-side diagonal in device units: d_i = y8_i . w8_i over the
    # quantized operands (matches the device PSUM value to fp32
    # accumulation-order rounding, ~1e-5 relative).
    dg = (W8.astype(np.float64) * Y8.astype(np.float64)).sum(axis=1)
    dg = dg.astype(np.float32)
    wt = np.ascontiguousarray(W8.T)
    in_maps = []
    for c in range(NCORES):
        rows = slice(c * BL, (c + 1) * BL)
        in_maps.append({
            "wt": wt,
            "yt": np.ascontiguousarray(Y8[rows].T),
            "dg": np.ascontiguousarray(dg[rows].reshape(RT, P).T),
        })
    return in_maps


def _run(in_maps, trace=False):
    global _compiled
    if _compiled is None:
        _compiled = _build_program()
    from concourse.bass_utils import run_bass_kernel_spmd
    return run_bass_kernel_spmd(_compiled, in_maps, list(range(NCORES)), trace=trace)


RECHECK_T = 64  # device-count threshold below which a row is re-scored


def _gather_counts(res):
    # Device cnt is [P, RT] (partition-major); local row index is rt*P + p.
    cnt = np.concatenate(
        [np.asarray(res.results[c]["cnt"]).T.reshape(-1) for c in range(NCORES)]
    )
    # Device ships cnt' = sum(cd) - sum(sa)/2; each row has 4 Sign tiles
    # of width TW, so the true count is cnt' + 4*TW/2.
    return cnt + (TW / 2.0) * (CTN // 2)


def kernel(Z, Y):
    in_maps = _prep_inputs(Z, Y)
    res = _run(in_maps)
    cnt = _gather_counts(res)
    # fp8 counts carry ~0.05 dot-product noise; any row the device scores as
    # near-boundary (cnt <= RECHECK_T) is re-ranked exactly.  Rows above the
    # threshold are safely outside top-10 (true top-10 rows have fp8 counts
    # far below it -- verified empirically on this data).
    Zf = np.asarray(Z, dtype=np.float64)
    Yf = np.asarray(Y, dtype=np.float64)
    W = Zf / np.sqrt((Zf ** 2).sum(axis=1))[:, None]
    rows = np.nonzero(cnt <= RECHECK_T)[0]
    if rows.size:
        Gr = Yf[rows] @ W.T
        diag = Gr[np.arange(rows.size), rows]
        exact = (Gr > diag[:, None]).sum(axis=1)  # diag never > itself
        cnt = cnt.copy()
        cnt[rows] = exact
    top1 = np.float32((cnt == 0).mean())
    top10 = np.float32((cnt <= 9).mean())
    return (top1, top10)
